# revision 11
# baseline (speedup 1.0000x reference)
"""Expert-parallel MoE (soft routing) kernel for 8 TRN2 NeuronCores — fp8 DoubleRow.

Problem (nn_EnhancedMixtureOfExperts): every expert processes the full batch,
outputs mixed by soft cluster probabilities.

    h1 = relu(x @ W1[e] + b1[e])      x:[B,D]  W1[e]:[D,H]
    h2 = relu(h1 @ W2[e] + b2[e])     W2[e]:[H,H2]
    y  = sigmoid(h2 @ W3[e] + b3[e])  W3[e]:[H2,1]
    out[b] = sum_e y[e,b] * probs[b,e]

Sharding: expert-parallel — core e computes expert e over the full batch.
The weighted combine is 8*B MACs, done on the host after gather.

Numerics: all GEMM operands are fp8 e4m3 (TRN FP8_EXP4; bit-compatible with
OCP e4m3fn for |v| <= 240), matmuls run perf_mode=DoubleRow (2 fp8 weights
per PE cell -> 256-row contraction per instruction, 2x bf16-rate).

GEMM3 (OUT=1) is folded away: with s = sum_k relu(pre2[k]) * w3[k] and
relu(a*z) = a*relu(z) for a>0,
    s = sum_k sign(w3k) * relu(pre2[k] * |w3k|),
so the host scales W2 columns by |w3| (and by a per-expert power-ish scale
SW2 to center the fp8 range) and GEMM2 is emitted "swapped" (h1 block as
the stationary operand, W2'' as moving) so its PSUM output lands
[batch, k]. A single fused Vector-engine scalar_tensor_tensor per PSUM tile
then computes relu (max 0) * sign with accum_out = the free-axis sum — the
whole former GEMM3 runs on the otherwise-idle DVE. b1/b3 stay exact;
nonzero b2 is handled by an optional DVE pre-add (the reference always has
b2 = 0 so the default build skips it).
"""

import numpy as np
import ml_dtypes

import concourse.bass as bass
import concourse.bacc as bacc
import concourse.mybir as mybir
from concourse.bass_utils import run_bass_kernel_spmd
from concourse.tile import TileContext

E = 8
B = 16384
D = 1024
H = 2048
H2 = 1024
NB = 512  # batch columns per chunk (one PSUM bank of fp32)

F32 = mybir.dt.float32
BF16 = mybir.dt.bfloat16
FP8 = mybir.dt.float8e4
AF = mybir.ActivationFunctionType
DR = mybir.MatmulPerfMode.DoubleRow
ALU = mybir.AluOpType

DBLK = D // 128   # 8
HBLK = H // 128   # 16
KBLK = H2 // 128  # 8
NBLK = NB // 128  # 4

SW1 = 1024.0      # host-side W1 scale (folded back out in the relu)
INV_SW1 = 1.0 / SW1

NP_FP8 = ml_dtypes.float8_e4m3fn


def build_moe_nc(batch: int = B, has_b2: bool = False) -> bass.Bass:
    nchunk = batch // NB
    nc = bacc.Bacc("TRN2")

    xT = nc.declare_dram_parameter("xT", [D, batch], FP8, isOutput=False)
    # w1[p, hb, db, c] = SW1 * W1[db*128+p, hb*128+c]
    w1 = nc.declare_dram_parameter("w1", [128, HBLK, DBLK, 128], FP8, isOutput=False)
    # w2[p, hb, k] = SW2_e * |w3[k]| * W2[hb*128+p, k]
    w2 = nc.declare_dram_parameter("w2", [128, HBLK, H2], FP8, isOutput=False)
    # sgn[p, k] = sign(w3[k]) (same for all partitions)
    sgn = nc.declare_dram_parameter("sgn", [128, H2], F32, isOutput=False)
    b1 = nc.declare_dram_parameter("b1", [128, HBLK], F32, isOutput=False)
    b3 = nc.declare_dram_parameter("b3", [128, 1], F32, isOutput=False)
    scl = nc.declare_dram_parameter("scl", [128, 1], F32, isOutput=False)  # 1/SW2_e
    if has_b2:
        b2a = nc.declare_dram_parameter("b2a", [128, H2], F32, isOutput=False)
    # y[p, cb] = out[cb*128 + p]
    y = nc.declare_dram_parameter("y", [128, batch // 128], F32, isOutput=True)

    with TileContext(nc) as tc:
        with (
            tc.tile_pool(name="wpool", bufs=1) as wpool,
            tc.tile_pool(name="xpool", bufs=3) as xpool,
            tc.tile_pool(name="h1pool", bufs=2) as h1pool,
            tc.tile_pool(name="scrpool", bufs=2) as scrpool,
            tc.tile_pool(name="accpool", bufs=8) as accpool,
            tc.tile_pool(name="ypool", bufs=4) as ypool,
            tc.tile_pool(name="pp1", bufs=3, space="PSUM") as pp1,
            tc.tile_pool(name="pp2", bufs=4, space="PSUM") as pp2,
        ):
            # Weights resident in SBUF for the whole kernel. All weight/const
            # loads issue from the (otherwise idle) GpSimd queue: each
            # DMA_DIRECT2D costs ~600ns of descriptor-gen on its issuing
            # engine, and serializing ~40 of them behind x(0) on Sync was
            # delaying the first matmul by ~7us.
            w1_sb = wpool.tile([128, HBLK, DBLK, 128], FP8)
            w2_sb = wpool.tile([128, HBLK, H2], FP8)
            sgn_sb = wpool.tile([128, H2], F32)
            b1_sb = wpool.tile([128, HBLK], F32)
            b3_sb = wpool.tile([128, 1], F32)
            scl_sb = wpool.tile([128, 1], F32)
            if has_b2:
                b2a_sb = wpool.tile([128, H2], F32)

            xT_r = xT.rearrange("(a p) (c n) -> p a c n", p=128, n=NB)

            for c in range(nchunk):
                x_sb = xpool.tile([128, DBLK, NB], FP8, name="x_sb")
                nc.sync.dma_start(out=x_sb, in_=xT_r[:, :, c, :])
                if c == 0:
                    # w1 per-hb slices (contiguous 1KB partition lines) so
                    # GEMM1 of chunk 0 starts once x(0) + slice 0 land;
                    # w2/sgn behind them on the same queue (first needed by
                    # GEMM2, ~halfway into chunk 0). Starting compute any
                    # earlier just trades into early-DMA-bandwidth stalls.
                    for hb in range(HBLK):
                        nc.sync.dma_start(
                            out=w1_sb[:, hb, :, :], in_=w1[:, hb, :, :]
                        )
                    nc.sync.dma_start(out=sgn_sb, in_=sgn[:, :])
                    nc.sync.dma_start(out=b1_sb, in_=b1[:, :])
                    nc.sync.dma_start(out=b3_sb, in_=b3[:, :])
                    nc.sync.dma_start(out=scl_sb, in_=scl[:, :])
                    if has_b2:
                        nc.sync.dma_start(out=b2a_sb, in_=b2a[:, :])
                    for hb in range(HBLK):
                        nc.sync.dma_start(
                            out=w2_sb[:, hb, :], in_=w2[:, hb, :]
                        )

                # GEMM1: h1T[h, b] = relu((W1*SW1).T @ xT) / SW1 + b1,
                # h on partitions.
                h1_sb = h1pool.tile([128, HBLK, NB], FP8, name="h1_sb")
                for hb in range(HBLK):
                    ps1 = pp1.tile([128, NB], F32, name="ps1")
                    for j in range(DBLK // 2):
                        nc.tensor.matmul(
                            ps1,
                            w1_sb[:, hb, 2 * j : 2 * j + 2, :],
                            x_sb[:, 2 * j : 2 * j + 2, :],
                            start=(j == 0),
                            stop=(j == DBLK // 2 - 1),
                            perf_mode=DR,
                        )
                    nc.scalar.activation(
                        h1_sb[:, hb, :], ps1, AF.Relu,
                        bias=b1_sb[:, hb : hb + 1], scale=INV_SW1,
                    )

                # GEMM2 (swapped): ps2[b, k] = h1_blk.T @ W2'' for each
                # 128-batch block and 512-k half; then one fused DVE op does
                # relu * sign and free-axis-accumulates into acc.
                y_sb = ypool.tile([128, NBLK], F32, name="y_sb")
                for blk in range(NBLK):
                    b0 = blk * 128
                    acc = accpool.tile([128, 3], F32, name="acc")
                    for half in range(2):
                        k0 = half * 512
                        ps2 = pp2.tile([128, 512], F32, name="ps2")
                        for j in range(HBLK // 2):
                            nc.tensor.matmul(
                                ps2,
                                h1_sb[:, 2 * j : 2 * j + 2, b0 : b0 + 128],
                                w2_sb[:, 2 * j : 2 * j + 2, k0 : k0 + 512],
                                start=(j == 0),
                                stop=(j == HBLK // 2 - 1),
                                perf_mode=DR,
                            )
                        if has_b2:
                            nc.vector.scalar_tensor_tensor(
                                out=ps2, in0=ps2, scalar=1.0,
                                in1=b2a_sb[:, k0 : k0 + 512],
                                op0=ALU.mult, op1=ALU.add,
                            )
                        scr = scrpool.tile([128, 512], BF16, name="scr")
                        nc.vector.scalar_tensor_tensor(
                            out=scr, in0=ps2, scalar=0.0,
                            in1=sgn_sb[:, k0 : k0 + 512],
                            op0=ALU.max, op1=ALU.mult,
                            accum_out=acc[:, half : half + 1],
                        )
                    nc.vector.scalar_tensor_tensor(
                        out=acc[:, 2:3], in0=acc[:, 0:1], scalar=0.0,
                        in1=acc[:, 1:2], op0=ALU.add, op1=ALU.add,
                    )
                    nc.scalar.activation(
                        y_sb[:, blk : blk + 1], acc[:, 2:3], AF.Sigmoid,
                        bias=b3_sb[:, 0:1], scale=scl_sb[:, 0:1],
                    )
                nc.sync.dma_start(
                    out=y[:, c * NBLK : (c + 1) * NBLK], in_=y_sb
                )

    nc.finalize()
    return nc


def to_fp8(a: np.ndarray) -> np.ndarray:
    return np.clip(np.asarray(a, dtype=np.float32), -240.0, 240.0).astype(NP_FP8)


def make_in_maps(
    x: np.ndarray,
    W1: np.ndarray,
    b1: np.ndarray,
    W2: np.ndarray,
    b2: np.ndarray,
    W3: np.ndarray,
    b3: np.ndarray,
) -> tuple[list[dict[str, np.ndarray]], bool]:
    xT = np.ascontiguousarray(to_fp8(np.asarray(x, dtype=np.float32).T))
    has_b2 = bool(np.any(np.asarray(b2)))
    in_maps = []
    for e in range(E):
        w1q = to_fp8(
            (np.asarray(W1[e], dtype=np.float32) * SW1)
            .reshape(DBLK, 128, HBLK, 128)
            .transpose(1, 2, 0, 3)
        )
        w3e = np.asarray(W3[e], dtype=np.float32).reshape(H2)
        w2ss = np.asarray(W2[e], dtype=np.float32) * np.abs(w3e)[None, :]  # [H, H2]
        m = float(np.max(np.abs(w2ss)))
        sw2 = 224.0 / m if m > 0 else 1.0
        w2q = to_fp8((w2ss * sw2).reshape(HBLK, 128, H2).transpose(1, 0, 2))
        sgn_row = np.sign(w3e).astype(np.float32)
        im = {
            "xT": xT,
            "w1": np.ascontiguousarray(w1q),
            "w2": np.ascontiguousarray(w2q),
            "sgn": np.ascontiguousarray(np.broadcast_to(sgn_row, (128, H2))),
            "b1": np.ascontiguousarray(
                np.asarray(b1[e], dtype=np.float32).reshape(HBLK, 128).T
            ),
            "b3": np.full((128, 1), np.asarray(b3[e], dtype=np.float32).reshape(()),
                          dtype=np.float32),
            "scl": np.full((128, 1), 1.0 / sw2, dtype=np.float32),
        }
        if has_b2:
            b2s = np.asarray(b2[e], dtype=np.float32) * np.abs(w3e) * sw2  # [H2]
            im["b2a"] = np.ascontiguousarray(
                np.broadcast_to(b2s.astype(np.float32), (128, H2))
            )
        in_maps.append(im)
    return in_maps, has_b2


_NC_CACHE: dict[tuple, bass.Bass] = {}


def run_on_hw(in_maps, batch: int = B, has_b2: bool = False, **kw):
    key = (batch, has_b2)
    nc = _NC_CACHE.get(key)
    if nc is None:
        nc = build_moe_nc(batch, has_b2)
        _NC_CACHE[key] = nc
    return run_bass_kernel_spmd(nc, in_maps, list(range(E)), **kw)


def kernel(x, soft_cluster_probs, W1, b1, W2, b2, W3, b3) -> np.ndarray:
    in_maps, has_b2 = make_in_maps(x, W1, b1, W2, b2, W3, b3)
    res = run_on_hw(in_maps, batch=x.shape[0], has_b2=has_b2)
    # y param [128, B/128]: out[cb*128 + p] = y[p, cb]
    y_all = np.stack(
        [res.results[e]["y"].T.reshape(-1) for e in range(E)], axis=0
    )  # [E, B]
    combined = np.einsum(
        "eb,be->b", y_all, np.asarray(soft_cluster_probs, dtype=np.float32)
    )
    return combined.astype(np.float32).reshape(-1, 1)


# revision 12
# speedup vs baseline: 1.0034x; 1.0034x over previous
"""Expert-parallel MoE (soft routing) kernel for 8 TRN2 NeuronCores — fp8 DoubleRow.

Problem (nn_EnhancedMixtureOfExperts): every expert processes the full batch,
outputs mixed by soft cluster probabilities.

    h1 = relu(x @ W1[e] + b1[e])      x:[B,D]  W1[e]:[D,H]
    h2 = relu(h1 @ W2[e] + b2[e])     W2[e]:[H,H2]
    y  = sigmoid(h2 @ W3[e] + b3[e])  W3[e]:[H2,1]
    out[b] = sum_e y[e,b] * probs[b,e]

Sharding: expert-parallel — core e computes expert e over the full batch.
The weighted combine is 8*B MACs, done on the host after gather.

Numerics: all GEMM operands are fp8 e4m3 (TRN FP8_EXP4; bit-compatible with
OCP e4m3fn for |v| <= 240), matmuls run perf_mode=DoubleRow (2 fp8 weights
per PE cell -> 256-row contraction per instruction, 2x bf16-rate).

GEMM3 (OUT=1) is folded away: with s = sum_k relu(pre2[k]) * w3[k] and
relu(a*z) = a*relu(z) for a>0,
    s = sum_k sign(w3k) * relu(pre2[k] * |w3k|),
so the host scales W2 columns by |w3| (and by a per-expert power-ish scale
SW2 to center the fp8 range) and GEMM2 is emitted "swapped" (h1 block as
the stationary operand, W2'' as moving) so its PSUM output lands
[batch, k]. A single fused Vector-engine scalar_tensor_tensor per PSUM tile
then computes relu (max 0) * sign with accum_out = the free-axis sum — the
whole former GEMM3 runs on the otherwise-idle DVE. b1/b3 stay exact;
nonzero b2 is handled by an optional DVE pre-add (the reference always has
b2 = 0 so the default build skips it).
"""

import numpy as np
import ml_dtypes

import concourse.bass as bass
import concourse.bacc as bacc
import concourse.mybir as mybir
from concourse.bass_utils import run_bass_kernel_spmd
from concourse.tile import TileContext

E = 8
B = 16384
D = 1024
H = 2048
H2 = 1024
NB = 512  # batch columns per chunk (one PSUM bank of fp32)

F32 = mybir.dt.float32
BF16 = mybir.dt.bfloat16
FP8 = mybir.dt.float8e4
AF = mybir.ActivationFunctionType
DR = mybir.MatmulPerfMode.DoubleRow
ALU = mybir.AluOpType

DBLK = D // 128   # 8
HBLK = H // 128   # 16
KBLK = H2 // 128  # 8
NBLK = NB // 128  # 4

SW1 = 1024.0      # host-side W1 scale (folded back out in the relu)
INV_SW1 = 1.0 / SW1

NP_FP8 = ml_dtypes.float8_e4m3fn


def build_moe_nc(batch: int = B, has_b2: bool = False) -> bass.Bass:
    nchunk = batch // NB
    nc = bacc.Bacc("TRN2")

    xT = nc.declare_dram_parameter("xT", [D, batch], FP8, isOutput=False)
    # w1[p, db, hb, c] = SW1 * W1[db*128+p, hb*128+c]
    w1 = nc.declare_dram_parameter("w1", [128, DBLK, HBLK, 128], FP8, isOutput=False)
    # w2[p, hb, k] = SW2_e * |w3[k]| * W2[hb*128+p, k]
    w2 = nc.declare_dram_parameter("w2", [128, HBLK, H2], FP8, isOutput=False)
    # sgn[p, k] = sign(w3[k]) (same for all partitions)
    sgn = nc.declare_dram_parameter("sgn", [128, H2], F32, isOutput=False)
    b1 = nc.declare_dram_parameter("b1", [128, HBLK], F32, isOutput=False)
    b3 = nc.declare_dram_parameter("b3", [128, 1], F32, isOutput=False)
    scl = nc.declare_dram_parameter("scl", [128, 1], F32, isOutput=False)  # 1/SW2_e
    if has_b2:
        b2a = nc.declare_dram_parameter("b2a", [128, H2], F32, isOutput=False)
    # y[p, cb] = out[cb*128 + p]
    y = nc.declare_dram_parameter("y", [128, batch // 128], F32, isOutput=True)

    with TileContext(nc) as tc:
        with (
            tc.tile_pool(name="wpool", bufs=1) as wpool,
            tc.tile_pool(name="xpool", bufs=3) as xpool,
            tc.tile_pool(name="h1pool", bufs=2) as h1pool,
            tc.tile_pool(name="scrpool", bufs=2) as scrpool,
            tc.tile_pool(name="accpool", bufs=8) as accpool,
            tc.tile_pool(name="ypool", bufs=4) as ypool,
            tc.tile_pool(name="pp1", bufs=3, space="PSUM") as pp1,
            tc.tile_pool(name="pp2", bufs=4, space="PSUM") as pp2,
        ):
            # Weights resident in SBUF for the whole kernel. All weight/const
            # loads issue from the (otherwise idle) GpSimd queue: each
            # DMA_DIRECT2D costs ~600ns of descriptor-gen on its issuing
            # engine, and serializing ~40 of them behind x(0) on Sync was
            # delaying the first matmul by ~7us.
            w1_sb = wpool.tile([128, DBLK, HBLK, 128], FP8)
            w2_sb = wpool.tile([128, HBLK, H2], FP8)
            sgn_sb = wpool.tile([128, H2], F32)
            b1_sb = wpool.tile([128, HBLK], F32)
            b3_sb = wpool.tile([128, 1], F32)
            scl_sb = wpool.tile([128, 1], F32)
            if has_b2:
                b2a_sb = wpool.tile([128, H2], F32)

            xT_r = xT.rearrange("(a p) (c n) -> p a c n", p=128, n=NB)

            for c in range(nchunk):
                x_sb = xpool.tile([128, DBLK, NB], FP8, name="x_sb")
                nc.sync.dma_start(out=x_sb, in_=xT_r[:, :, c, :])
                if c == 0:
                    # w1 in db-pair slices (contiguous 4KB partition lines)
                    # so GEMM1 of chunk 0 starts once x(0) + slice 0 land;
                    # w2/sgn behind them on the same queue (first needed by
                    # GEMM2, ~halfway into chunk 0). Starting compute any
                    # earlier just trades into early-DMA-bandwidth stalls.
                    for j in range(DBLK // 2):
                        nc.sync.dma_start(
                            out=w1_sb[:, 2 * j : 2 * j + 2, :, :],
                            in_=w1[:, 2 * j : 2 * j + 2, :, :],
                        )
                    nc.sync.dma_start(out=sgn_sb, in_=sgn[:, :])
                    nc.sync.dma_start(out=b1_sb, in_=b1[:, :])
                    nc.sync.dma_start(out=b3_sb, in_=b3[:, :])
                    nc.sync.dma_start(out=scl_sb, in_=scl[:, :])
                    if has_b2:
                        nc.sync.dma_start(out=b2a_sb, in_=b2a[:, :])
                    for hb in range(HBLK):
                        nc.sync.dma_start(
                            out=w2_sb[:, hb, :], in_=w2[:, hb, :]
                        )

                # GEMM1: h1T[h, b] = relu((W1*SW1).T @ xT) / SW1 + b1,
                # h on partitions.
                h1_sb = h1pool.tile([128, HBLK, NB], FP8, name="h1_sb")
                for hb in range(HBLK):
                    ps1 = pp1.tile([128, NB], F32, name="ps1")
                    for j in range(DBLK // 2):
                        nc.tensor.matmul(
                            ps1,
                            w1_sb[:, 2 * j : 2 * j + 2, hb, :],
                            x_sb[:, 2 * j : 2 * j + 2, :],
                            start=(j == 0),
                            stop=(j == DBLK // 2 - 1),
                            perf_mode=DR,
                        )
                    nc.scalar.activation(
                        h1_sb[:, hb, :], ps1, AF.Relu,
                        bias=b1_sb[:, hb : hb + 1], scale=INV_SW1,
                    )

                # GEMM2 (swapped): ps2[b, k] = h1_blk.T @ W2'' for each
                # 128-batch block and 512-k half; then one fused DVE op does
                # relu * sign and free-axis-accumulates into acc.
                y_sb = ypool.tile([128, NBLK], F32, name="y_sb")
                for blk in range(NBLK):
                    b0 = blk * 128
                    acc = accpool.tile([128, 3], F32, name="acc")
                    for half in range(2):
                        k0 = half * 512
                        ps2 = pp2.tile([128, 512], F32, name="ps2")
                        for j in range(HBLK // 2):
                            nc.tensor.matmul(
                                ps2,
                                h1_sb[:, 2 * j : 2 * j + 2, b0 : b0 + 128],
                                w2_sb[:, 2 * j : 2 * j + 2, k0 : k0 + 512],
                                start=(j == 0),
                                stop=(j == HBLK // 2 - 1),
                                perf_mode=DR,
                            )
                        if has_b2:
                            nc.vector.scalar_tensor_tensor(
                                out=ps2, in0=ps2, scalar=1.0,
                                in1=b2a_sb[:, k0 : k0 + 512],
                                op0=ALU.mult, op1=ALU.add,
                            )
                        scr = scrpool.tile([128, 512], BF16, name="scr")
                        nc.vector.scalar_tensor_tensor(
                            out=scr, in0=ps2, scalar=0.0,
                            in1=sgn_sb[:, k0 : k0 + 512],
                            op0=ALU.max, op1=ALU.mult,
                            accum_out=acc[:, half : half + 1],
                        )
                    nc.vector.scalar_tensor_tensor(
                        out=acc[:, 2:3], in0=acc[:, 0:1], scalar=0.0,
                        in1=acc[:, 1:2], op0=ALU.add, op1=ALU.add,
                    )
                    nc.scalar.activation(
                        y_sb[:, blk : blk + 1], acc[:, 2:3], AF.Sigmoid,
                        bias=b3_sb[:, 0:1], scale=scl_sb[:, 0:1],
                    )
                nc.sync.dma_start(
                    out=y[:, c * NBLK : (c + 1) * NBLK], in_=y_sb
                )

    nc.finalize()
    return nc


def to_fp8(a: np.ndarray) -> np.ndarray:
    return np.clip(np.asarray(a, dtype=np.float32), -240.0, 240.0).astype(NP_FP8)


def make_in_maps(
    x: np.ndarray,
    W1: np.ndarray,
    b1: np.ndarray,
    W2: np.ndarray,
    b2: np.ndarray,
    W3: np.ndarray,
    b3: np.ndarray,
) -> tuple[list[dict[str, np.ndarray]], bool]:
    xT = np.ascontiguousarray(to_fp8(np.asarray(x, dtype=np.float32).T))
    has_b2 = bool(np.any(np.asarray(b2)))
    in_maps = []
    for e in range(E):
        w1q = to_fp8(
            (np.asarray(W1[e], dtype=np.float32) * SW1)
            .reshape(DBLK, 128, HBLK, 128)
            .transpose(1, 0, 2, 3)
        )
        w3e = np.asarray(W3[e], dtype=np.float32).reshape(H2)
        w2ss = np.asarray(W2[e], dtype=np.float32) * np.abs(w3e)[None, :]  # [H, H2]
        m = float(np.max(np.abs(w2ss)))
        sw2 = 224.0 / m if m > 0 else 1.0
        w2q = to_fp8((w2ss * sw2).reshape(HBLK, 128, H2).transpose(1, 0, 2))
        sgn_row = np.sign(w3e).astype(np.float32)
        im = {
            "xT": xT,
            "w1": np.ascontiguousarray(w1q),
            "w2": np.ascontiguousarray(w2q),
            "sgn": np.ascontiguousarray(np.broadcast_to(sgn_row, (128, H2))),
            "b1": np.ascontiguousarray(
                np.asarray(b1[e], dtype=np.float32).reshape(HBLK, 128).T
            ),
            "b3": np.full((128, 1), np.asarray(b3[e], dtype=np.float32).reshape(()),
                          dtype=np.float32),
            "scl": np.full((128, 1), 1.0 / sw2, dtype=np.float32),
        }
        if has_b2:
            b2s = np.asarray(b2[e], dtype=np.float32) * np.abs(w3e) * sw2  # [H2]
            im["b2a"] = np.ascontiguousarray(
                np.broadcast_to(b2s.astype(np.float32), (128, H2))
            )
        in_maps.append(im)
    return in_maps, has_b2


_NC_CACHE: dict[tuple, bass.Bass] = {}


def run_on_hw(in_maps, batch: int = B, has_b2: bool = False, **kw):
    key = (batch, has_b2)
    nc = _NC_CACHE.get(key)
    if nc is None:
        nc = build_moe_nc(batch, has_b2)
        _NC_CACHE[key] = nc
    return run_bass_kernel_spmd(nc, in_maps, list(range(E)), **kw)


def kernel(x, soft_cluster_probs, W1, b1, W2, b2, W3, b3) -> np.ndarray:
    in_maps, has_b2 = make_in_maps(x, W1, b1, W2, b2, W3, b3)
    res = run_on_hw(in_maps, batch=x.shape[0], has_b2=has_b2)
    # y param [128, B/128]: out[cb*128 + p] = y[p, cb]
    y_all = np.stack(
        [res.results[e]["y"].T.reshape(-1) for e in range(E)], axis=0
    )  # [E, B]
    combined = np.einsum(
        "eb,be->b", y_all, np.asarray(soft_cluster_probs, dtype=np.float32)
    )
    return combined.astype(np.float32).reshape(-1, 1)
